# revision 30
# baseline (speedup 1.0000x reference)
"""Trainium2 Bass kernel for the non-local attention block (nn_CPP_80676665688885).

Sharding: pure data-parallel over batch — 1 sample per NeuronCore (B=8, 8 cores).
BatchNorm batch-statistics are combined with a tiny (2 KB) AllReduce.

All matmuls run in float32r (single-pass, 1 col/cycle like bf16, ~13-bit
mantissa) — no hi/lo splitting needed anywhere. exp() output stays in f32r.
Softmax normalization (1/s) is deferred past the W-conv: (Ww@y)/s == Ww@(y/s),
so the per-tile critical chain is fT -> exp -> y -> wconv with the reciprocal
path (s -> 1/s -> partition-broadcast via K=1 matmul) running in parallel.

Per-core algorithm (sample x: (C=256, N=4096), N = 64x64 spatial):
  theta = Wt@x + bt                       phi,g = maxpool2(conv) + bias post-pool
  per n-tile (512 cols), per m-chunk (128):
    fT   = phi^T @ theta   (f32r)  ; ef = exp(fT) (f32r, ScalarE)
    y   += gT^T @ ef ; s += ones^T @ ef
  r = 1/s ; rb = broadcast(r) via ones(1,128)^T @ r matmul
  wy_n = (Ww @ y) * rb  -> fp16, with fused Σ (BN s1); Σwy² via gpsimd+vector
  stats AllReduce over 8 cores ; scale = gamma*rsqrt(var+eps) (rsqrt = exp(-½ln))
  out[c] = max_n(wy_n*scale + x) + (beta - mean*scale)
"""

import os
import sys

import numpy as np
from contextlib import ExitStack

for _p in ("/opt/trn_rl_repo",):
    if os.path.isdir(_p) and _p not in sys.path:
        sys.path.append(_p)

import concourse.bass as bass
import concourse.bacc as bacc
import concourse.tile as tile
from concourse import mybir
from concourse.bass_utils import run_bass_kernel_spmd

F32 = mybir.dt.float32
F32R = mybir.dt.float32r
F16 = mybir.dt.float16
BF16 = mybir.dt.bfloat16
AF = mybir.ActivationFunctionType
ALU = mybir.AluOpType
AX = mybir.AxisListType

B = 8
C = 256
CI = 128
N = 4096          # 64*64
M = 1024          # 32*32 after 2x2 maxpool
NT = 512          # n-tile (PSUM bank width in fp32)
NTILES = N // NT  # 8
MCH = M // 128    # 8 m-chunks
CCH = C // 128    # 2 channel chunks
EPS = 1e-5
INV_CNT = 1.0 / (B * N)

_CACHE = {}


def _build():
    nc = bacc.Bacc("TRN2", num_devices=B)

    x_d = nc.declare_dram_parameter("x", [C, N], BF16, False)
    w_d = {}
    for nm in ("t", "p", "g"):
        w_d[nm] = nc.declare_dram_parameter(f"W{nm}T", [C, CI], BF16, False)
    wwT_d = nc.declare_dram_parameter("WwT", [CI, C], BF16, False)
    smalls_d = nc.declare_dram_parameter("smalls", [128, 7], F32, False)
    ones_k_d = nc.declare_dram_parameter("ones_k", [128, 1], BF16, False)
    ones_p_d = nc.declare_dram_parameter("ones_p", [1, 128], BF16, False)
    out_d = nc.declare_dram_parameter("out", [CCH, 128], F32, True)

    ident_d = nc.inline_tensor(np.eye(128, dtype=np.float32), name="ident")

    warm_in = nc.dram_tensor("warm_in", [1, 8], F32)
    warm2_out = nc.dram_tensor("warm2_out", [1, 8], F32, addr_space="Shared")
    ka16_d = nc.dram_tensor("ka16", [1, 8], F16)
    warm_out = nc.dram_tensor("warm_out", [1, 8], F32, addr_space="Shared")
    stats_in = nc.dram_tensor("stats_in", [128, 2 * CCH], F32)
    stats_out = nc.dram_tensor("stats_out", [128, 2 * CCH], F32,
                               addr_space="Shared")

    with ExitStack() as ctx:
        tc = ctx.enter_context(tile.TileContext(nc))
        consts = ctx.enter_context(tc.tile_pool(name="consts", bufs=1))
        persist = ctx.enter_context(tc.tile_pool(name="persist", bufs=1))
        efp = ctx.enter_context(tc.tile_pool(name="efp", bufs=4))
        sm = ctx.enter_context(tc.tile_pool(name="sm", bufs=2))
        small = ctx.enter_context(tc.tile_pool(name="small", bufs=4))
        # PSUM: 8 banks total
        ps_ft = ctx.enter_context(tc.tile_pool(name="ps_ft", bufs=3, space="PSUM"))
        ps_y = ctx.enter_context(tc.tile_pool(name="ps_y", bufs=2, space="PSUM"))
        ps_s = ctx.enter_context(tc.tile_pool(name="ps_s", bufs=1, space="PSUM"))
        ps_mix = ctx.enter_context(tc.tile_pool(name="ps_mix", bufs=2, space="PSUM"))

        # ---- x DMAs first (sync queue), weights on scalar queue, consts on vector ----
        x_sb = [persist.tile([128, N], BF16, tag=f"x{ch}", name=f"x_sb{ch}")
                for ch in range(CCH)]
        QN = N // 4
        for q in range(4):
            qs = slice(q * QN, (q + 1) * QN)
            for ch in range(CCH):
                nc.sync.dma_start(out=x_sb[ch][:, qs],
                                  in_=x_d[ch * 128:(ch + 1) * 128, qs])

        w_sb = {}
        for nm in ("t", "p", "g"):
            w_sb[nm] = consts.tile([128, CCH, CI], BF16, name=f"w_{nm}")
            nc.scalar.dma_start(
                out=w_sb[nm][:, :, :],
                in_=w_d[nm].rearrange("(c2 p) ci -> p c2 ci", p=128))
        ww_sb = consts.tile([128, CCH, 128], BF16)
        nc.scalar.dma_start(out=ww_sb[:, :, :],
                            in_=wwT_d.rearrange("p (c2 k) -> p c2 k", c2=CCH))
        smalls = consts.tile([128, 7], F32)
        nc.scalar.dma_start(out=smalls, in_=smalls_d[:, :])
        bt_sb = smalls[:, 0:1]
        bp_sb = smalls[:, 1:2]
        bg_sb = smalls[:, 2:3]
        gamma_sb = smalls[:, 3:5]
        beta_sb = smalls[:, 5:7]

        ident = consts.tile([128, 128], F32)
        nc.gpsimd.dma_start(out=ident, in_=ident_d[:, :])
        ones_k = consts.tile([128, 1], BF16)
        nc.gpsimd.dma_start(out=ones_k, in_=ones_k_d[:, :])
        ones_p = consts.tile([1, 128], BF16)
        nc.gpsimd.dma_start(out=ones_p, in_=ones_p_d[:, :])
        eps_sb = consts.tile([128, 1], F32)
        nc.vector.memset(eps_sb, EPS)

        # warm up the collective path early (overlaps with compute)
        warm_sb = small.tile([1, 8], F32, tag="warm")
        nc.vector.memset(warm_sb, 1.0)
        nc.gpsimd.dma_start(out=warm_in[:, :], in_=warm_sb)
        nc.gpsimd.collective_compute(
            "AllReduce", ALU.add, replica_groups=[list(range(B))],
            ins=[warm_in[:, :]], outs=[warm_out[:, :]])

        # fp16 copy of x for the finale (vector, overlaps conv matmuls)
        x16 = [persist.tile([128, N], F16, tag=f"x16_{ch}", name=f"x16_{ch}")
               for ch in range(CCH)]

        # ---- projections ----
        wy16 = [persist.tile([128, N], F16, tag=f"wy{ch}", name=f"wy16_{ch}")
                for ch in range(CCH)]
        s1p = persist.tile([128, CCH, NTILES], F32, tag="s1p")
        s2p = persist.tile([128, CCH, NTILES], F32, tag="s2p")
        th = persist.tile([128, N], BF16, tag="th")
        phi_pool = persist.tile([128, M], BF16, tag="phip")
        g_pool = persist.tile([128, M], F32, tag="gp")
        gT = persist.tile([128, MCH, CI], BF16, tag="gT")

        def conv_mms(ps, nm, sl):
            for ch in range(CCH):
                nc.tensor.matmul(ps, lhsT=w_sb[nm][:, ch, :], rhs=x_sb[ch][:, sl],
                                 start=(ch == 0), stop=(ch == CCH - 1))

        def conv_theta(it):
            tsl = slice(it * NT, (it + 1) * NT)
            ps = ps_ft.tile([128, NT], F32, tag="ft", name="theta_ps")
            conv_mms(ps, "t", tsl)
            nc.scalar.activation(out=th[:, tsl], in_=ps, func=AF.Identity,
                                 bias=bt_sb, scale=1.0)

        def attn_mc(ait, mc, yps, sps):
            asl = slice(ait * NT, (ait + 1) * NT)
            ams = slice(mc * 128, (mc + 1) * 128)
            fps = ps_ft.tile([128, NT], F32, tag="ft", name="fps")
            nc.tensor.matmul(fps, lhsT=phi_pool[:, ams], rhs=th[:, asl],
                             start=True, stop=True)
            ef = efp.tile([128, NT], BF16, tag="ef", name="ef")
            nc.scalar.activation(out=ef, in_=fps, func=AF.Exp)
            nc.tensor.matmul(yps, lhsT=gT[:, mc, :], rhs=ef,
                             start=(mc == 0), stop=(mc == MCH - 1))
            nc.tensor.matmul(sps, lhsT=ones_k, rhs=ef,
                             start=(mc == 0), stop=(mc == MCH - 1))

        conv_theta(0)
        yps0 = ps_y.tile([128, NT], F32, tag="yps", name="yps0")
        sps0 = ps_s.tile([1, NT], F32, tag="sps", name="sps0")
        for it in range(NTILES):
            sl = slice(it * NT, (it + 1) * NT)
            ms = slice(it * 128, (it + 1) * 128)
            # phi conv -> copy -> 2x2 maxpool -> +bias
            psp = ps_ft.tile([128, NT], F32, tag="ft")
            conv_mms(psp, "p", sl)
            pcp = sm.tile([128, NT], F32, tag="pcp")
            nc.scalar.copy(out=pcp, in_=psp)
            pr = pcp.rearrange("p (h wp t) -> p h wp t", h=8, wp=32, t=2)
            pm = sm.tile([128, 8, 32], F32, tag="pm")
            nc.vector.tensor_tensor(out=pm, in0=pr[:, :, :, 0], in1=pr[:, :, :, 1],
                                    op=ALU.max)
            pm2 = pm.rearrange("p (hp s) wp -> p hp s wp", s=2)
            pp = sm.tile([128, 128], F32, tag="pp")
            nc.vector.tensor_tensor(
                out=pp.rearrange("p (hp wp) -> p hp wp", hp=4),
                in0=pm2[:, :, 0, :], in1=pm2[:, :, 1, :], op=ALU.max)
            nc.vector.tensor_scalar_add(out=phi_pool[:, ms], in0=pp,
                                        scalar1=bp_sb)
            # g conv -> copy -> maxpool -> +bias -> transpose
            psg = ps_ft.tile([128, NT], F32, tag="ft")
            conv_mms(psg, "g", sl)
            gcp = sm.tile([128, NT], F32, tag="gcp")
            nc.vector.tensor_copy(out=gcp, in_=psg)
            gr = gcp.rearrange("p (h wp t) -> p h wp t", h=8, wp=32, t=2)
            gm = sm.tile([128, 8, 32], F32, tag="gm")
            nc.vector.tensor_tensor(out=gm, in0=gr[:, :, :, 0], in1=gr[:, :, :, 1],
                                    op=ALU.max)
            gm2 = gm.rearrange("p (hp s) wp -> p hp s wp", s=2)
            nc.vector.tensor_tensor(
                out=g_pool[:, ms].rearrange("p (hp wp) -> p hp wp", hp=4),
                in0=gm2[:, :, 0, :], in1=gm2[:, :, 1, :], op=ALU.max)
            nc.vector.tensor_scalar_add(out=g_pool[:, ms], in0=g_pool[:, ms],
                                        scalar1=bg_sb)
            tp = ps_mix.tile([128, 128], F32, tag="mix")
            nc.tensor.transpose(tp, g_pool[:, ms], ident)
            nc.scalar.copy(out=gT[:, it, :], in_=tp)
            # x16 copies interleaved (vector)
            for ch in range(CCH):
                nc.vector.tensor_copy(out=x16[ch][:, sl], in_=x_sb[ch][:, sl])
            # attention for tile 0 rides inside phase 1 (chunk it-1 is ready)
            if it >= 1:
                attn_mc(0, it - 1, yps0, sps0)
            if it == 6:
                conv_theta(1)

        # ---- attention + W-conv, per n-tile ----

        # Software-pipelined: the boundary tensor work of tile it (rb
        # broadcast + W-conv + normalize/stats) is issued in the middle of
        # tile it+1's mc-loop so the tensor queue never stalls waiting for
        # the vector-side reciprocal chain.
        pend = {}

        def boundary_tensor(pit):
            rbps = ps_mix.tile([128, NT], F32, tag="mix")
            nc.tensor.matmul(rbps, lhsT=ones_p, rhs=pend["rr"],
                             start=True, stop=True)
            rb_sb = sm.tile([128, NT], F32, tag="rbsb")
            nc.vector.tensor_copy(out=rb_sb, in_=rbps)
            wps = {}
            for ch in range(CCH):
                wps[ch] = ps_mix.tile([128, NT], F32, tag="mix",
                                      name=f"wps{ch}")
                nc.tensor.matmul(wps[ch], lhsT=ww_sb[:, ch, :],
                                 rhs=pend["y_sb"], start=True, stop=True)
            pend["rb_sb"] = rb_sb
            pend["wps"] = wps

        def boundary_vector(pit):
            psl = slice(pit * NT, (pit + 1) * NT)
            rb_sb = pend["rb_sb"]
            wps = pend["wps"]
            for ch in range(CCH):
                # wy_n = wps * rb  (fp16), fused BN s1 accumulation
                nc.vector.scalar_tensor_tensor(
                    out=wy16[ch][:, psl], in0=wps[ch], scalar=1.0, in1=rb_sb,
                    op0=ALU.mult, op1=ALU.mult,
                    accum_out=s1p[:, ch, pit:pit + 1])
                sq = sm.tile([128, NT], F32, tag="sq")
                nc.vector.tensor_tensor(out=sq, in0=wy16[ch][:, psl],
                                        in1=wy16[ch][:, psl], op=ALU.mult)
                nc.vector.tensor_reduce(out=s2p[:, ch, pit:pit + 1], in_=sq,
                                        axis=AX.X, op=ALU.add)

        def tile_tail(it, yps, sps):
            # reciprocal (vector) + bf16 y copy-out; consumed by boundary()
            r32 = small.tile([1, NT], F32, tag="r32")
            nc.vector.reciprocal_approx_fast(out=r32, in_=sps)
            rr = small.tile([1, NT], BF16, tag="rr")
            nc.vector.tensor_copy(out=rr, in_=r32)
            y_sb = sm.tile([128, NT], BF16, tag="ysb")
            nc.vector.tensor_copy(out=y_sb, in_=yps)
            if it > 0:
                boundary_vector(it - 1)
            return {"rr": rr, "y_sb": y_sb}

        attn_mc(0, MCH - 1, yps0, sps0)
        pend = tile_tail(0, yps0, sps0)

        for it in range(1, NTILES):
            yps = ps_y.tile([128, NT], F32, tag="yps")
            sps = ps_s.tile([1, NT], F32, tag="sps")
            for mc in range(MCH):
                attn_mc(it, mc, yps, sps)
                if mc == 2:
                    boundary_tensor(it - 1)
                if mc == 5 and it < NTILES - 1:
                    conv_theta(it + 1)
                if mc == 7 and it == 5:
                    nc.gpsimd.collective_compute(
                        "AllReduce", ALU.add,
                        replica_groups=[list(range(B))],
                        ins=[warm_in[:, :]], outs=[warm2_out[:, :]])
            pend = tile_tail(it, yps, sps)

        boundary_tensor(NTILES - 1)
        boundary_vector(NTILES - 1)

        # ---- combine partials, AllReduce, finalize ----
        stats_sb = small.tile([128, 2 * CCH], F32, tag="stats")
        nc.vector.tensor_reduce(out=stats_sb[:, 0:2], in_=s1p[:, :, :],
                                axis=AX.X, op=ALU.add)
        nc.vector.tensor_reduce(out=stats_sb[:, 2:4], in_=s2p[:, :, :],
                                axis=AX.X, op=ALU.add)
        nc.gpsimd.dma_start(out=stats_in[:, :], in_=stats_sb)
        nc.gpsimd.collective_compute(
            "AllReduce", ALU.add, replica_groups=[list(range(B))],
            ins=[stats_in[:, :]], outs=[stats_out[:, :]])
        stats_g = small.tile([128, 2 * CCH], F32, tag="statsg")
        nc.gpsimd.dma_start(out=stats_g, in_=stats_out[:, :])

        out_sb = small.tile([128, CCH], F32, tag="outsb")
        mean2 = small.tile([128, CCH], F32, tag="fin")
        e22 = small.tile([128, CCH], F32, tag="fin")
        var2 = small.tile([128, CCH], F32, tag="fin")
        nc.vector.tensor_scalar_mul(out=mean2, in0=stats_g[:, 0:2],
                                    scalar1=INV_CNT)
        nc.vector.tensor_scalar_mul(out=e22, in0=stats_g[:, 2:4],
                                    scalar1=INV_CNT)
        m22 = small.tile([128, CCH], F32, tag="fin")
        nc.scalar.square(out=m22, in_=mean2)
        nc.vector.tensor_tensor(out=var2, in0=e22, in1=m22, op=ALU.subtract)
        sd2 = small.tile([128, CCH], F32, tag="fin")
        nc.scalar.activation(out=sd2, in_=var2, func=AF.Sqrt, bias=eps_sb,
                             scale=1.0)
        rstd2 = small.tile([128, CCH], F32, tag="fin")
        nc.vector.reciprocal_approx_fast(out=rstd2, in_=sd2)
        scale2 = small.tile([128, CCH], F32, tag="fin")
        nc.vector.tensor_tensor(out=scale2, in0=rstd2, in1=gamma_sb,
                                op=ALU.mult)
        ms2 = small.tile([128, CCH], F32, tag="fin")
        nc.vector.tensor_tensor(out=ms2, in0=mean2, in1=scale2, op=ALU.mult)
        negshift2 = small.tile([128, CCH], F32, tag="fin")
        nc.vector.tensor_tensor(out=negshift2, in0=ms2, in1=beta_sb,
                                op=ALU.subtract)

        for ch in range(CCH):
            # z = wy16*scale + x16 ; out = max_n z - negshift
            z = sm.tile([128, N], F16, tag=f"z{ch}")
            nc.vector.scalar_tensor_tensor(out=z, in0=wy16[ch][:, :],
                                           scalar=scale2[:, ch:ch + 1],
                                           in1=x16[ch][:, :], op0=ALU.mult,
                                           op1=ALU.add)
            mx = small.tile([128, 1], F16, tag="finh")
            nc.vector.tensor_reduce(out=mx, in_=z, axis=AX.X, op=ALU.max)
            nc.vector.tensor_tensor(out=out_sb[:, ch:ch + 1], in0=mx,
                                    in1=negshift2[:, ch:ch + 1],
                                    op=ALU.subtract)
            nc.gpsimd.dma_start(out=out_d[ch, :].rearrange("(p one) -> p one", one=1),
                                in_=out_sb[:, ch:ch + 1])

    nc.compile()
    return nc


_LAST = {}


def _to_bf16(a):
    try:
        import ml_dtypes
        return np.ascontiguousarray(a.astype(ml_dtypes.bfloat16))
    except ImportError:
        import jax.numpy as jnp
        return np.ascontiguousarray(np.asarray(jnp.asarray(a, dtype=jnp.bfloat16)))


def kernel(**inputs):
    x = np.ascontiguousarray(inputs["x"], dtype=np.float32)      # (8, 256, 64, 64)
    Wg = np.asarray(inputs["Wg"], dtype=np.float32)
    bg = np.asarray(inputs["bg"], dtype=np.float32)
    Wt = np.asarray(inputs["Wt"], dtype=np.float32)
    bt = np.asarray(inputs["bt"], dtype=np.float32)
    Wp = np.asarray(inputs["Wp"], dtype=np.float32)
    bp = np.asarray(inputs["bp"], dtype=np.float32)
    Ww = np.asarray(inputs["Ww"], dtype=np.float32)
    bw = np.asarray(inputs["bw"], dtype=np.float32)
    gamma = np.asarray(inputs["gamma"], dtype=np.float32)
    beta = np.asarray(inputs["beta"], dtype=np.float32)

    if "nc" not in _CACHE:
        _CACHE["nc"] = _build()
    nc = _CACHE["nc"]

    shared = {
        "WtT": _to_bf16(np.ascontiguousarray(Wt.T)),
        "WpT": _to_bf16(np.ascontiguousarray(Wp.T)),
        "WgT": _to_bf16(np.ascontiguousarray(Wg.T)),
        "WwT": _to_bf16(np.ascontiguousarray(Ww.T)),
        "smalls": np.ascontiguousarray(np.concatenate([
            bt.reshape(CI, 1), bp.reshape(CI, 1), bg.reshape(CI, 1),
            gamma.reshape(CCH, 128).T, beta.reshape(CCH, 128).T,
        ], axis=1).astype(np.float32)),
        "ones_k": _to_bf16(np.ones((128, 1), dtype=np.float32)),
        "ones_p": _to_bf16(np.ones((1, 128), dtype=np.float32)),
    }
    in_maps = [dict(shared, x=_to_bf16(x[b].reshape(C, N)))
               for b in range(B)]
    import os
    trace = bool(int(os.environ.get("KERNEL_TRACE", "0")))
    res = run_bass_kernel_spmd(nc, in_maps, core_ids=list(range(B)), trace=trace)
    _LAST["res"] = res
    out = np.stack([np.asarray(res.results[b]["out"]).reshape(C) for b in range(B)])
    return out.reshape(B, C, 1, 1).astype(np.float32)


if __name__ == "__main__":
    pass


# revision 31
# speedup vs baseline: 1.0270x; 1.0270x over previous
"""Trainium2 Bass kernel for the non-local attention block (nn_CPP_80676665688885).

Sharding: pure data-parallel over batch — 1 sample per NeuronCore (B=8, 8 cores).
BatchNorm batch-statistics are combined with a tiny (2 KB) AllReduce.

All matmuls run in float32r (single-pass, 1 col/cycle like bf16, ~13-bit
mantissa) — no hi/lo splitting needed anywhere. exp() output stays in f32r.
Softmax normalization (1/s) is deferred past the W-conv: (Ww@y)/s == Ww@(y/s),
so the per-tile critical chain is fT -> exp -> y -> wconv with the reciprocal
path (s -> 1/s -> partition-broadcast via K=1 matmul) running in parallel.

Per-core algorithm (sample x: (C=256, N=4096), N = 64x64 spatial):
  theta = Wt@x + bt                       phi,g = maxpool2(conv) + bias post-pool
  per n-tile (512 cols), per m-chunk (128):
    fT   = phi^T @ theta   (f32r)  ; ef = exp(fT) (f32r, ScalarE)
    y   += gT^T @ ef ; s += ones^T @ ef
  r = 1/s ; rb = broadcast(r) via ones(1,128)^T @ r matmul
  wy_n = (Ww @ y) * rb  -> fp16, with fused Σ (BN s1); Σwy² via gpsimd+vector
  stats AllReduce over 8 cores ; scale = gamma*rsqrt(var+eps) (rsqrt = exp(-½ln))
  out[c] = max_n(wy_n*scale + x) + (beta - mean*scale)
"""

import os
import sys

import numpy as np
from contextlib import ExitStack

for _p in ("/opt/trn_rl_repo",):
    if os.path.isdir(_p) and _p not in sys.path:
        sys.path.append(_p)

import concourse.bass as bass
import concourse.bacc as bacc
import concourse.tile as tile
from concourse import mybir
from concourse.bass_utils import run_bass_kernel_spmd

F32 = mybir.dt.float32
F32R = mybir.dt.float32r
F16 = mybir.dt.float16
BF16 = mybir.dt.bfloat16
AF = mybir.ActivationFunctionType
ALU = mybir.AluOpType
AX = mybir.AxisListType

B = 8
C = 256
CI = 128
N = 4096          # 64*64
M = 1024          # 32*32 after 2x2 maxpool
NT = 512          # n-tile (PSUM bank width in fp32)
NTILES = N // NT  # 8
MCH = M // 128    # 8 m-chunks
CCH = C // 128    # 2 channel chunks
EPS = 1e-5
INV_CNT = 1.0 / (B * N)

_CACHE = {}


def _build():
    nc = bacc.Bacc("TRN2", num_devices=B)

    x_d = nc.declare_dram_parameter("x", [C, N], BF16, False)
    w_d = {}
    for nm in ("t", "p", "g"):
        w_d[nm] = nc.declare_dram_parameter(f"W{nm}T", [C, CI], BF16, False)
    wwT_d = nc.declare_dram_parameter("WwT", [CI, C], BF16, False)
    smalls_d = nc.declare_dram_parameter("smalls", [128, 7], F32, False)
    ones_k_d = nc.declare_dram_parameter("ones_k", [128, 1], BF16, False)
    ones_p_d = nc.declare_dram_parameter("ones_p", [1, 128], BF16, False)
    out_d = nc.declare_dram_parameter("out", [CCH, 128], F32, True)

    ident_d = nc.inline_tensor(np.eye(128, dtype=np.float32), name="ident")

    warm_in = nc.dram_tensor("warm_in", [1, 8], F32)
    ka16_d = nc.dram_tensor("ka16", [1, 8], F16)
    warm_out = nc.dram_tensor("warm_out", [1, 8], F32, addr_space="Shared")
    stats_in = nc.dram_tensor("stats_in", [128, 2 * CCH], F32)
    stats_out = nc.dram_tensor("stats_out", [128, 2 * CCH], F32,
                               addr_space="Shared")

    with ExitStack() as ctx:
        tc = ctx.enter_context(tile.TileContext(nc))
        consts = ctx.enter_context(tc.tile_pool(name="consts", bufs=1))
        persist = ctx.enter_context(tc.tile_pool(name="persist", bufs=1))
        efp = ctx.enter_context(tc.tile_pool(name="efp", bufs=4))
        sm = ctx.enter_context(tc.tile_pool(name="sm", bufs=2))
        small = ctx.enter_context(tc.tile_pool(name="small", bufs=4))
        # PSUM: 8 banks total
        ps_ft = ctx.enter_context(tc.tile_pool(name="ps_ft", bufs=3, space="PSUM"))
        ps_y = ctx.enter_context(tc.tile_pool(name="ps_y", bufs=2, space="PSUM"))
        ps_s = ctx.enter_context(tc.tile_pool(name="ps_s", bufs=1, space="PSUM"))
        ps_mix = ctx.enter_context(tc.tile_pool(name="ps_mix", bufs=2, space="PSUM"))

        # ---- x DMAs first (sync queue), weights on scalar queue, consts on vector ----
        x_sb = [persist.tile([128, N], BF16, tag=f"x{ch}", name=f"x_sb{ch}")
                for ch in range(CCH)]
        QN = N // 4
        for q in range(4):
            qs = slice(q * QN, (q + 1) * QN)
            for ch in range(CCH):
                nc.sync.dma_start(out=x_sb[ch][:, qs],
                                  in_=x_d[ch * 128:(ch + 1) * 128, qs])

        w_sb = {}
        for nm in ("t", "p", "g"):
            w_sb[nm] = consts.tile([128, CCH, CI], BF16, name=f"w_{nm}")
            nc.scalar.dma_start(
                out=w_sb[nm][:, :, :],
                in_=w_d[nm].rearrange("(c2 p) ci -> p c2 ci", p=128))
        ww_sb = consts.tile([128, CCH, 128], BF16)
        nc.scalar.dma_start(out=ww_sb[:, :, :],
                            in_=wwT_d.rearrange("p (c2 k) -> p c2 k", c2=CCH))
        smalls = consts.tile([128, 7], F32)
        nc.scalar.dma_start(out=smalls, in_=smalls_d[:, :])
        bt_sb = smalls[:, 0:1]
        bp_sb = smalls[:, 1:2]
        bg_sb = smalls[:, 2:3]
        gamma_sb = smalls[:, 3:5]
        beta_sb = smalls[:, 5:7]

        ident = consts.tile([128, 128], F32)
        nc.gpsimd.dma_start(out=ident, in_=ident_d[:, :])
        ones_k = consts.tile([128, 1], BF16)
        nc.gpsimd.dma_start(out=ones_k, in_=ones_k_d[:, :])
        ones_p = consts.tile([1, 128], BF16)
        nc.gpsimd.dma_start(out=ones_p, in_=ones_p_d[:, :])
        eps_sb = consts.tile([128, 1], F32)
        nc.vector.memset(eps_sb, EPS)

        # warm up the collective path early (overlaps with compute)
        warm_sb = small.tile([1, 8], F32, tag="warm")
        nc.vector.memset(warm_sb, 1.0)
        nc.gpsimd.dma_start(out=warm_in[:, :], in_=warm_sb)
        nc.gpsimd.collective_compute(
            "AllReduce", ALU.add, replica_groups=[list(range(B))],
            ins=[warm_in[:, :]], outs=[warm_out[:, :]])

        # fp16 copy of x for the finale (vector, overlaps conv matmuls)
        x16 = [persist.tile([128, N], F16, tag=f"x16_{ch}", name=f"x16_{ch}")
               for ch in range(CCH)]

        # ---- projections ----
        wy16 = [persist.tile([128, N], F16, tag=f"wy{ch}", name=f"wy16_{ch}")
                for ch in range(CCH)]
        s1p = persist.tile([128, CCH, NTILES], F32, tag="s1p")
        s2p = persist.tile([128, CCH, NTILES], F32, tag="s2p")
        th = persist.tile([128, N], BF16, tag="th")
        phi_pool = persist.tile([128, M], BF16, tag="phip")
        g_pool = persist.tile([128, M], F32, tag="gp")
        gT = persist.tile([128, MCH, CI], BF16, tag="gT")

        def conv_mms(ps, nm, sl):
            for ch in range(CCH):
                nc.tensor.matmul(ps, lhsT=w_sb[nm][:, ch, :], rhs=x_sb[ch][:, sl],
                                 start=(ch == 0), stop=(ch == CCH - 1))

        def conv_theta(it):
            tsl = slice(it * NT, (it + 1) * NT)
            ps = ps_ft.tile([128, NT], F32, tag="ft", name="theta_ps")
            conv_mms(ps, "t", tsl)
            nc.scalar.activation(out=th[:, tsl], in_=ps, func=AF.Identity,
                                 bias=bt_sb, scale=1.0)

        def attn_mc(ait, mc, yps, sps):
            asl = slice(ait * NT, (ait + 1) * NT)
            ams = slice(mc * 128, (mc + 1) * 128)
            fps = ps_ft.tile([128, NT], F32, tag="ft", name="fps")
            nc.tensor.matmul(fps, lhsT=phi_pool[:, ams], rhs=th[:, asl],
                             start=True, stop=True)
            ef = efp.tile([128, NT], BF16, tag="ef", name="ef")
            nc.scalar.activation(out=ef, in_=fps, func=AF.Exp)
            nc.tensor.matmul(yps, lhsT=gT[:, mc, :], rhs=ef,
                             start=(mc == 0), stop=(mc == MCH - 1))
            nc.tensor.matmul(sps, lhsT=ones_k, rhs=ef,
                             start=(mc == 0), stop=(mc == MCH - 1))

        conv_theta(0)
        yps0 = ps_y.tile([128, NT], F32, tag="yps", name="yps0")
        sps0 = ps_s.tile([1, NT], F32, tag="sps", name="sps0")
        for it in range(NTILES):
            sl = slice(it * NT, (it + 1) * NT)
            ms = slice(it * 128, (it + 1) * 128)
            # phi conv -> copy -> 2x2 maxpool -> +bias
            psp = ps_ft.tile([128, NT], F32, tag="ft")
            conv_mms(psp, "p", sl)
            pcp = sm.tile([128, NT], F32, tag="pcp")
            nc.scalar.copy(out=pcp, in_=psp)
            pr = pcp.rearrange("p (h wp t) -> p h wp t", h=8, wp=32, t=2)
            pm = sm.tile([128, 8, 32], F32, tag="pm")
            nc.vector.tensor_tensor(out=pm, in0=pr[:, :, :, 0], in1=pr[:, :, :, 1],
                                    op=ALU.max)
            pm2 = pm.rearrange("p (hp s) wp -> p hp s wp", s=2)
            pp = sm.tile([128, 128], F32, tag="pp")
            nc.vector.tensor_tensor(
                out=pp.rearrange("p (hp wp) -> p hp wp", hp=4),
                in0=pm2[:, :, 0, :], in1=pm2[:, :, 1, :], op=ALU.max)
            nc.vector.tensor_scalar_add(out=phi_pool[:, ms], in0=pp,
                                        scalar1=bp_sb)
            # g conv -> copy -> maxpool -> +bias -> transpose
            psg = ps_ft.tile([128, NT], F32, tag="ft")
            conv_mms(psg, "g", sl)
            gcp = sm.tile([128, NT], F32, tag="gcp")
            nc.vector.tensor_copy(out=gcp, in_=psg)
            gr = gcp.rearrange("p (h wp t) -> p h wp t", h=8, wp=32, t=2)
            gm = sm.tile([128, 8, 32], F32, tag="gm")
            nc.vector.tensor_tensor(out=gm, in0=gr[:, :, :, 0], in1=gr[:, :, :, 1],
                                    op=ALU.max)
            gm2 = gm.rearrange("p (hp s) wp -> p hp s wp", s=2)
            nc.vector.tensor_tensor(
                out=g_pool[:, ms].rearrange("p (hp wp) -> p hp wp", hp=4),
                in0=gm2[:, :, 0, :], in1=gm2[:, :, 1, :], op=ALU.max)
            nc.vector.tensor_scalar_add(out=g_pool[:, ms], in0=g_pool[:, ms],
                                        scalar1=bg_sb)
            tp = ps_mix.tile([128, 128], F32, tag="mix")
            nc.tensor.transpose(tp, g_pool[:, ms], ident)
            nc.scalar.copy(out=gT[:, it, :], in_=tp)
            # x16 copies interleaved (vector)
            for ch in range(CCH):
                nc.vector.tensor_copy(out=x16[ch][:, sl], in_=x_sb[ch][:, sl])
            # attention for tile 0 rides inside phase 1 (chunk it-1 is ready)
            if it >= 1:
                attn_mc(0, it - 1, yps0, sps0)
            if it == 6:
                conv_theta(1)

        # ---- attention + W-conv, per n-tile ----

        # Software-pipelined: the boundary tensor work of tile it (rb
        # broadcast + W-conv + normalize/stats) is issued in the middle of
        # tile it+1's mc-loop so the tensor queue never stalls waiting for
        # the vector-side reciprocal chain.
        pend = {}

        def boundary_tensor(pit):
            rbps = ps_mix.tile([128, NT], F32, tag="mix")
            nc.tensor.matmul(rbps, lhsT=ones_p, rhs=pend["rr"],
                             start=True, stop=True)
            rb_sb = sm.tile([128, NT], F32, tag="rbsb")
            nc.vector.tensor_copy(out=rb_sb, in_=rbps)
            wps = {}
            for ch in range(CCH):
                wps[ch] = ps_mix.tile([128, NT], F32, tag="mix",
                                      name=f"wps{ch}")
                nc.tensor.matmul(wps[ch], lhsT=ww_sb[:, ch, :],
                                 rhs=pend["y_sb"], start=True, stop=True)
            pend["rb_sb"] = rb_sb
            pend["wps"] = wps

        def boundary_vector(pit):
            psl = slice(pit * NT, (pit + 1) * NT)
            rb_sb = pend["rb_sb"]
            wps = pend["wps"]
            for ch in range(CCH):
                # wy_n = wps * rb  (fp16), fused BN s1 accumulation
                nc.vector.scalar_tensor_tensor(
                    out=wy16[ch][:, psl], in0=wps[ch], scalar=1.0, in1=rb_sb,
                    op0=ALU.mult, op1=ALU.mult,
                    accum_out=s1p[:, ch, pit:pit + 1])
                sq = sm.tile([128, NT], F32, tag="sq")
                nc.vector.tensor_tensor(out=sq, in0=wy16[ch][:, psl],
                                        in1=wy16[ch][:, psl], op=ALU.mult)
                nc.vector.tensor_reduce(out=s2p[:, ch, pit:pit + 1], in_=sq,
                                        axis=AX.X, op=ALU.add)

        def tile_tail(it, yps, sps):
            # reciprocal (vector) + bf16 y copy-out; consumed by boundary()
            r32 = small.tile([1, NT], F32, tag="r32")
            nc.vector.reciprocal_approx_fast(out=r32, in_=sps)
            rr = small.tile([1, NT], BF16, tag="rr")
            nc.vector.tensor_copy(out=rr, in_=r32)
            y_sb = sm.tile([128, NT], BF16, tag="ysb")
            nc.vector.tensor_copy(out=y_sb, in_=yps)
            if it > 0:
                boundary_vector(it - 1)
            return {"rr": rr, "y_sb": y_sb}

        attn_mc(0, MCH - 1, yps0, sps0)
        pend = tile_tail(0, yps0, sps0)

        for it in range(1, NTILES):
            yps = ps_y.tile([128, NT], F32, tag="yps")
            sps = ps_s.tile([1, NT], F32, tag="sps")
            for mc in range(MCH):
                attn_mc(it, mc, yps, sps)
                if mc == 2:
                    boundary_tensor(it - 1)
                if mc == 5 and it < NTILES - 1:
                    conv_theta(it + 1)
            pend = tile_tail(it, yps, sps)

        boundary_tensor(NTILES - 1)
        boundary_vector(NTILES - 1)

        # ---- combine partials, AllReduce, finalize ----
        stats_sb = small.tile([128, 2 * CCH], F32, tag="stats")
        nc.vector.tensor_reduce(out=stats_sb[:, 0:2], in_=s1p[:, :, :],
                                axis=AX.X, op=ALU.add)
        nc.vector.tensor_reduce(out=stats_sb[:, 2:4], in_=s2p[:, :, :],
                                axis=AX.X, op=ALU.add)
        nc.gpsimd.dma_start(out=stats_in[:, :], in_=stats_sb)
        nc.gpsimd.collective_compute(
            "AllReduce", ALU.add, replica_groups=[list(range(B))],
            ins=[stats_in[:, :]], outs=[stats_out[:, :]])
        stats_g = small.tile([128, 2 * CCH], F32, tag="statsg")
        nc.gpsimd.dma_start(out=stats_g, in_=stats_out[:, :])

        out_sb = small.tile([128, CCH], F32, tag="outsb")
        mean2 = small.tile([128, CCH], F32, tag="fin")
        e22 = small.tile([128, CCH], F32, tag="fin")
        var2 = small.tile([128, CCH], F32, tag="fin")
        nc.vector.tensor_scalar_mul(out=mean2, in0=stats_g[:, 0:2],
                                    scalar1=INV_CNT)
        nc.vector.tensor_scalar_mul(out=e22, in0=stats_g[:, 2:4],
                                    scalar1=INV_CNT)
        m22 = small.tile([128, CCH], F32, tag="fin")
        nc.scalar.square(out=m22, in_=mean2)
        nc.vector.tensor_tensor(out=var2, in0=e22, in1=m22, op=ALU.subtract)
        sd2 = small.tile([128, CCH], F32, tag="fin")
        nc.scalar.activation(out=sd2, in_=var2, func=AF.Sqrt, bias=eps_sb,
                             scale=1.0)
        rstd2 = small.tile([128, CCH], F32, tag="fin")
        nc.vector.reciprocal_approx_fast(out=rstd2, in_=sd2)
        scale2 = small.tile([128, CCH], F32, tag="fin")
        nc.vector.tensor_tensor(out=scale2, in0=rstd2, in1=gamma_sb,
                                op=ALU.mult)
        ms2 = small.tile([128, CCH], F32, tag="fin")
        nc.vector.tensor_tensor(out=ms2, in0=mean2, in1=scale2, op=ALU.mult)
        negshift2 = small.tile([128, CCH], F32, tag="fin")
        nc.vector.tensor_tensor(out=negshift2, in0=ms2, in1=beta_sb,
                                op=ALU.subtract)

        for ch in range(CCH):
            # z = wy16*scale + x16 ; out = max_n z - negshift
            z = sm.tile([128, N], F16, tag=f"z{ch}")
            nc.vector.scalar_tensor_tensor(out=z, in0=wy16[ch][:, :],
                                           scalar=scale2[:, ch:ch + 1],
                                           in1=x16[ch][:, :], op0=ALU.mult,
                                           op1=ALU.add)
            mx = small.tile([128, 1], F16, tag="finh")
            nc.vector.tensor_reduce(out=mx, in_=z, axis=AX.X, op=ALU.max)
            nc.vector.tensor_tensor(out=out_sb[:, ch:ch + 1], in0=mx,
                                    in1=negshift2[:, ch:ch + 1],
                                    op=ALU.subtract)
            nc.gpsimd.dma_start(out=out_d[ch, :].rearrange("(p one) -> p one", one=1),
                                in_=out_sb[:, ch:ch + 1])

    nc.compile()
    return nc


_LAST = {}


def _to_bf16(a):
    try:
        import ml_dtypes
        return np.ascontiguousarray(a.astype(ml_dtypes.bfloat16))
    except ImportError:
        import jax.numpy as jnp
        return np.ascontiguousarray(np.asarray(jnp.asarray(a, dtype=jnp.bfloat16)))


def kernel(**inputs):
    x = np.ascontiguousarray(inputs["x"], dtype=np.float32)      # (8, 256, 64, 64)
    Wg = np.asarray(inputs["Wg"], dtype=np.float32)
    bg = np.asarray(inputs["bg"], dtype=np.float32)
    Wt = np.asarray(inputs["Wt"], dtype=np.float32)
    bt = np.asarray(inputs["bt"], dtype=np.float32)
    Wp = np.asarray(inputs["Wp"], dtype=np.float32)
    bp = np.asarray(inputs["bp"], dtype=np.float32)
    Ww = np.asarray(inputs["Ww"], dtype=np.float32)
    bw = np.asarray(inputs["bw"], dtype=np.float32)
    gamma = np.asarray(inputs["gamma"], dtype=np.float32)
    beta = np.asarray(inputs["beta"], dtype=np.float32)

    if "nc" not in _CACHE:
        _CACHE["nc"] = _build()
    nc = _CACHE["nc"]

    shared = {
        "WtT": _to_bf16(np.ascontiguousarray(Wt.T)),
        "WpT": _to_bf16(np.ascontiguousarray(Wp.T)),
        "WgT": _to_bf16(np.ascontiguousarray(Wg.T)),
        "WwT": _to_bf16(np.ascontiguousarray(Ww.T)),
        "smalls": np.ascontiguousarray(np.concatenate([
            bt.reshape(CI, 1), bp.reshape(CI, 1), bg.reshape(CI, 1),
            gamma.reshape(CCH, 128).T, beta.reshape(CCH, 128).T,
        ], axis=1).astype(np.float32)),
        "ones_k": _to_bf16(np.ones((128, 1), dtype=np.float32)),
        "ones_p": _to_bf16(np.ones((1, 128), dtype=np.float32)),
    }
    in_maps = [dict(shared, x=_to_bf16(x[b].reshape(C, N)))
               for b in range(B)]
    import os
    trace = bool(int(os.environ.get("KERNEL_TRACE", "0")))
    res = run_bass_kernel_spmd(nc, in_maps, core_ids=list(range(B)), trace=trace)
    _LAST["res"] = res
    out = np.stack([np.asarray(res.results[b]["out"]).reshape(C) for b in range(B)])
    return out.reshape(B, C, 1, 1).astype(np.float32)


if __name__ == "__main__":
    pass


# revision 32
# speedup vs baseline: 1.1081x; 1.0790x over previous
"""Trainium2 Bass kernel for the non-local attention block (nn_CPP_80676665688885).

Sharding: pure data-parallel over batch — 1 sample per NeuronCore (B=8, 8 cores).
BatchNorm batch-statistics are combined with a tiny (2 KB) AllReduce.

All matmuls run in float32r (single-pass, 1 col/cycle like bf16, ~13-bit
mantissa) — no hi/lo splitting needed anywhere. exp() output stays in f32r.
Softmax normalization (1/s) is deferred past the W-conv: (Ww@y)/s == Ww@(y/s),
so the per-tile critical chain is fT -> exp -> y -> wconv with the reciprocal
path (s -> 1/s -> partition-broadcast via K=1 matmul) running in parallel.

Per-core algorithm (sample x: (C=256, N=4096), N = 64x64 spatial):
  theta = Wt@x + bt                       phi,g = maxpool2(conv) + bias post-pool
  per n-tile (512 cols), per m-chunk (128):
    fT   = phi^T @ theta   (f32r)  ; ef = exp(fT) (f32r, ScalarE)
    y   += gT^T @ ef ; s += ones^T @ ef
  r = 1/s ; rb = broadcast(r) via ones(1,128)^T @ r matmul
  wy_n = (Ww @ y) * rb  -> fp16, with fused Σ (BN s1); Σwy² via gpsimd+vector
  stats AllReduce over 8 cores ; scale = gamma*rsqrt(var+eps) (rsqrt = exp(-½ln))
  out[c] = max_n(wy_n*scale + x) + (beta - mean*scale)
"""

import os
import sys

import numpy as np
from contextlib import ExitStack

for _p in ("/opt/trn_rl_repo",):
    if os.path.isdir(_p) and _p not in sys.path:
        sys.path.append(_p)

import concourse.bass as bass
import concourse.bacc as bacc
import concourse.tile as tile
from concourse import mybir
from concourse.bass_utils import run_bass_kernel_spmd

F32 = mybir.dt.float32
F32R = mybir.dt.float32r
F16 = mybir.dt.float16
BF16 = mybir.dt.bfloat16
AF = mybir.ActivationFunctionType
ALU = mybir.AluOpType
AX = mybir.AxisListType

B = 8
C = 256
CI = 128
N = 4096          # 64*64
M = 1024          # 32*32 after 2x2 maxpool
NT = 512          # n-tile (PSUM bank width in fp32)
NTILES = N // NT  # 8
MCH = M // 128    # 8 m-chunks
CCH = C // 128    # 2 channel chunks
EPS = 1e-5
INV_CNT = 1.0 / (B * N)

_CACHE = {}


def _build():
    nc = bacc.Bacc("TRN2", num_devices=B)

    x_d = nc.declare_dram_parameter("x", [C, N], BF16, False)
    w_d = {}
    for nm in ("t", "p", "g"):
        w_d[nm] = nc.declare_dram_parameter(f"W{nm}T", [C, CI], BF16, False)
    wwT_d = nc.declare_dram_parameter("WwT", [CI, C], BF16, False)
    smalls_d = nc.declare_dram_parameter("smalls", [128, 7], F32, False)
    ones_k_d = nc.declare_dram_parameter("ones_k", [128, 1], BF16, False)
    ones_p_d = nc.declare_dram_parameter("ones_p", [1, 128], BF16, False)
    out_d = nc.declare_dram_parameter("out", [CCH, 128], F32, True)

    ident_d = nc.inline_tensor(np.eye(128, dtype=np.float32), name="ident")

    warm_in = nc.dram_tensor("warm_in", [1, 8], F32)
    ka16_d = nc.dram_tensor("ka16", [1, 8], F16)
    warm_out = nc.dram_tensor("warm_out", [1, 8], F32, addr_space="Shared")
    stats_in = nc.dram_tensor("stats_in", [128, 2 * CCH], F32)
    stats_out = nc.dram_tensor("stats_out", [128, 2 * CCH], F32,
                               addr_space="Shared")

    with ExitStack() as ctx:
        tc = ctx.enter_context(tile.TileContext(nc))
        consts = ctx.enter_context(tc.tile_pool(name="consts", bufs=1))
        persist = ctx.enter_context(tc.tile_pool(name="persist", bufs=1))
        efp = ctx.enter_context(tc.tile_pool(name="efp", bufs=4))
        sm = ctx.enter_context(tc.tile_pool(name="sm", bufs=2))
        small = ctx.enter_context(tc.tile_pool(name="small", bufs=4))
        # PSUM: 8 banks total
        ps_ft = ctx.enter_context(tc.tile_pool(name="ps_ft", bufs=4, space="PSUM"))
        ps_y = ctx.enter_context(tc.tile_pool(name="ps_y", bufs=1, space="PSUM"))
        ps_s = ctx.enter_context(tc.tile_pool(name="ps_s", bufs=1, space="PSUM"))
        ps_mix = ctx.enter_context(tc.tile_pool(name="ps_mix", bufs=2, space="PSUM"))

        # ---- x DMAs first (sync queue), weights on scalar queue, consts on vector ----
        x_sb = [persist.tile([128, N], BF16, tag=f"x{ch}", name=f"x_sb{ch}")
                for ch in range(CCH)]
        pieces = [(0, 512), (512, 1024), (1024, 2048), (2048, 3072),
                  (3072, 4096)]
        for a, b in pieces:
            qs = slice(a, b)
            for ch in range(CCH):
                nc.sync.dma_start(out=x_sb[ch][:, qs],
                                  in_=x_d[ch * 128:(ch + 1) * 128, qs])

        w_sb = {}
        for nm in ("t", "p", "g"):
            w_sb[nm] = consts.tile([128, CCH, CI], BF16, name=f"w_{nm}")
            nc.scalar.dma_start(
                out=w_sb[nm][:, :, :],
                in_=w_d[nm].rearrange("(c2 p) ci -> p c2 ci", p=128))
        ww_sb = consts.tile([128, CCH, 128], BF16)
        nc.scalar.dma_start(out=ww_sb[:, :, :],
                            in_=wwT_d.rearrange("p (c2 k) -> p c2 k", c2=CCH))
        smalls = consts.tile([128, 7], F32)
        nc.scalar.dma_start(out=smalls, in_=smalls_d[:, :])
        bt_sb = smalls[:, 0:1]
        bp_sb = smalls[:, 1:2]
        bg_sb = smalls[:, 2:3]
        gamma_sb = smalls[:, 3:5]
        beta_sb = smalls[:, 5:7]

        ident = consts.tile([128, 128], F32)
        nc.gpsimd.dma_start(out=ident, in_=ident_d[:, :])
        ones_k = consts.tile([128, 1], BF16)
        nc.gpsimd.dma_start(out=ones_k, in_=ones_k_d[:, :])
        ones_p = consts.tile([1, 128], BF16)
        nc.gpsimd.dma_start(out=ones_p, in_=ones_p_d[:, :])
        eps_sb = consts.tile([128, 1], F32)
        nc.vector.memset(eps_sb, EPS)

        # warm up the collective path early (overlaps with compute)
        warm_sb = small.tile([1, 8], F32, tag="warm")
        nc.vector.memset(warm_sb, 1.0)
        nc.gpsimd.dma_start(out=warm_in[:, :], in_=warm_sb)
        nc.gpsimd.collective_compute(
            "AllReduce", ALU.add, replica_groups=[list(range(B))],
            ins=[warm_in[:, :]], outs=[warm_out[:, :]])

        # fp16 copy of x for the finale (vector, overlaps conv matmuls)
        x16 = [persist.tile([128, N], F16, tag=f"x16_{ch}", name=f"x16_{ch}")
               for ch in range(CCH)]

        # ---- projections ----
        wy16 = [persist.tile([128, N], F16, tag=f"wy{ch}", name=f"wy16_{ch}")
                for ch in range(CCH)]
        s1p = persist.tile([128, CCH, NTILES], F32, tag="s1p")
        s2p = persist.tile([128, CCH, NTILES], F32, tag="s2p")
        th = persist.tile([128, N], BF16, tag="th")
        phi_pool = persist.tile([128, M], BF16, tag="phip")
        g_pool = persist.tile([128, M], F32, tag="gp")
        gT = persist.tile([128, MCH, CI], BF16, tag="gT")

        def conv_mms(ps, nm, sl):
            for ch in range(CCH):
                nc.tensor.matmul(ps, lhsT=w_sb[nm][:, ch, :], rhs=x_sb[ch][:, sl],
                                 start=(ch == 0), stop=(ch == CCH - 1))

        def conv_theta(it):
            tsl = slice(it * NT, (it + 1) * NT)
            ps = ps_ft.tile([128, NT], F32, tag="ft", name="theta_ps")
            conv_mms(ps, "t", tsl)
            nc.scalar.activation(out=th[:, tsl], in_=ps, func=AF.Identity,
                                 bias=bt_sb, scale=1.0)

        def attn_mc(ait, mc, yps, sps):
            asl = slice(ait * NT, (ait + 1) * NT)
            ams = slice(mc * 128, (mc + 1) * 128)
            fps = ps_ft.tile([128, NT], F32, tag="ft", name="fps")
            nc.tensor.matmul(fps, lhsT=phi_pool[:, ams], rhs=th[:, asl],
                             start=True, stop=True)
            ef = efp.tile([128, NT], BF16, tag="ef", name="ef")
            nc.scalar.activation(out=ef, in_=fps, func=AF.Exp)
            nc.tensor.matmul(yps, lhsT=gT[:, mc, :], rhs=ef,
                             start=(mc == 0), stop=(mc == MCH - 1))
            nc.tensor.matmul(sps, lhsT=ones_k, rhs=ef,
                             start=(mc == 0), stop=(mc == MCH - 1))

        conv_theta(0)
        yps0 = ps_y.tile([128, NT], F32, tag="yps", name="yps0")
        sps0 = ps_s.tile([1, NT], F32, tag="sps", name="sps0")
        for it in range(NTILES):
            sl = slice(it * NT, (it + 1) * NT)
            ms = slice(it * 128, (it + 1) * 128)
            # phi conv -> copy -> 2x2 maxpool -> +bias
            psp = ps_ft.tile([128, NT], F32, tag="ft")
            conv_mms(psp, "p", sl)
            pcp = sm.tile([128, NT], F32, tag="pcp")
            nc.scalar.copy(out=pcp, in_=psp)
            pr = pcp.rearrange("p (h wp t) -> p h wp t", h=8, wp=32, t=2)
            pm = sm.tile([128, 8, 32], F32, tag="pm")
            nc.vector.tensor_tensor(out=pm, in0=pr[:, :, :, 0], in1=pr[:, :, :, 1],
                                    op=ALU.max)
            pm2 = pm.rearrange("p (hp s) wp -> p hp s wp", s=2)
            pp = sm.tile([128, 128], F32, tag="pp")
            nc.vector.tensor_tensor(
                out=pp.rearrange("p (hp wp) -> p hp wp", hp=4),
                in0=pm2[:, :, 0, :], in1=pm2[:, :, 1, :], op=ALU.max)
            nc.vector.tensor_scalar_add(out=phi_pool[:, ms], in0=pp,
                                        scalar1=bp_sb)
            # g conv -> copy -> maxpool -> +bias -> transpose
            psg = ps_ft.tile([128, NT], F32, tag="ft")
            conv_mms(psg, "g", sl)
            gcp = sm.tile([128, NT], F32, tag="gcp")
            nc.vector.tensor_copy(out=gcp, in_=psg)
            gr = gcp.rearrange("p (h wp t) -> p h wp t", h=8, wp=32, t=2)
            gm = sm.tile([128, 8, 32], F32, tag="gm")
            nc.vector.tensor_tensor(out=gm, in0=gr[:, :, :, 0], in1=gr[:, :, :, 1],
                                    op=ALU.max)
            gm2 = gm.rearrange("p (hp s) wp -> p hp s wp", s=2)
            nc.vector.tensor_tensor(
                out=g_pool[:, ms].rearrange("p (hp wp) -> p hp wp", hp=4),
                in0=gm2[:, :, 0, :], in1=gm2[:, :, 1, :], op=ALU.max)
            nc.vector.tensor_scalar_add(out=g_pool[:, ms], in0=g_pool[:, ms],
                                        scalar1=bg_sb)
            tp = ps_mix.tile([128, 128], F32, tag="mix")
            nc.tensor.transpose(tp, g_pool[:, ms], ident)
            nc.scalar.copy(out=gT[:, it, :], in_=tp)
            # x16 copies interleaved (vector)
            for ch in range(CCH):
                nc.vector.tensor_copy(out=x16[ch][:, sl], in_=x_sb[ch][:, sl])
            # attention for tile 0 rides inside phase 1 (chunk it-1 is ready)
            if it >= 1:
                attn_mc(0, it - 1, yps0, sps0)
            if it == 6:
                conv_theta(1)

        # ---- attention + W-conv, per n-tile ----

        # Software-pipelined: the boundary tensor work of tile it (rb
        # broadcast + W-conv + normalize/stats) is issued in the middle of
        # tile it+1's mc-loop so the tensor queue never stalls waiting for
        # the vector-side reciprocal chain.
        pend = {}

        def boundary_tensor(pit):
            rbps = ps_mix.tile([128, NT], F32, tag="mix")
            nc.tensor.matmul(rbps, lhsT=ones_p, rhs=pend["rr"],
                             start=True, stop=True)
            rb_sb = sm.tile([128, NT], F32, tag="rbsb")
            nc.vector.tensor_copy(out=rb_sb, in_=rbps)
            wps = {}
            for ch in range(CCH):
                wps[ch] = ps_mix.tile([128, NT], F32, tag="mix",
                                      name=f"wps{ch}")
                nc.tensor.matmul(wps[ch], lhsT=ww_sb[:, ch, :],
                                 rhs=pend["y_sb"], start=True, stop=True)
            pend["rb_sb"] = rb_sb
            pend["wps"] = wps

        def boundary_vector(pit):
            psl = slice(pit * NT, (pit + 1) * NT)
            rb_sb = pend["rb_sb"]
            wps = pend["wps"]
            for ch in range(CCH):
                # wy_n = wps * rb  (fp16), fused BN s1 accumulation
                nc.vector.scalar_tensor_tensor(
                    out=wy16[ch][:, psl], in0=wps[ch], scalar=1.0, in1=rb_sb,
                    op0=ALU.mult, op1=ALU.mult,
                    accum_out=s1p[:, ch, pit:pit + 1])
                sq = sm.tile([128, NT], F32, tag="sq")
                nc.vector.tensor_tensor(out=sq, in0=wy16[ch][:, psl],
                                        in1=wy16[ch][:, psl], op=ALU.mult)
                nc.vector.tensor_reduce(out=s2p[:, ch, pit:pit + 1], in_=sq,
                                        axis=AX.X, op=ALU.add)

        def tile_tail(it, yps, sps):
            # reciprocal (vector) + bf16 y copy-out; consumed by boundary()
            r32 = small.tile([1, NT], F32, tag="r32")
            nc.vector.reciprocal_approx_fast(out=r32, in_=sps)
            rr = small.tile([1, NT], BF16, tag="rr")
            nc.vector.tensor_copy(out=rr, in_=r32)
            y_sb = sm.tile([128, NT], BF16, tag="ysb")
            nc.vector.tensor_copy(out=y_sb, in_=yps)
            if it > 0:
                boundary_vector(it - 1)
            return {"rr": rr, "y_sb": y_sb}

        attn_mc(0, MCH - 1, yps0, sps0)
        pend = tile_tail(0, yps0, sps0)

        for it in range(1, NTILES):
            yps = ps_y.tile([128, NT], F32, tag="yps")
            sps = ps_s.tile([1, NT], F32, tag="sps")
            for mc in range(MCH):
                attn_mc(it, mc, yps, sps)
                if mc == 2:
                    boundary_tensor(it - 1)
                if mc == 5 and it < NTILES - 1:
                    conv_theta(it + 1)
            pend = tile_tail(it, yps, sps)

        boundary_tensor(NTILES - 1)
        boundary_vector(NTILES - 1)

        # ---- combine partials, AllReduce, finalize ----
        stats_sb = small.tile([128, 2 * CCH], F32, tag="stats")
        nc.vector.tensor_reduce(out=stats_sb[:, 0:2], in_=s1p[:, :, :],
                                axis=AX.X, op=ALU.add)
        nc.vector.tensor_reduce(out=stats_sb[:, 2:4], in_=s2p[:, :, :],
                                axis=AX.X, op=ALU.add)
        nc.gpsimd.dma_start(out=stats_in[:, :], in_=stats_sb)
        nc.gpsimd.collective_compute(
            "AllReduce", ALU.add, replica_groups=[list(range(B))],
            ins=[stats_in[:, :]], outs=[stats_out[:, :]])
        stats_g = small.tile([128, 2 * CCH], F32, tag="statsg")
        nc.gpsimd.dma_start(out=stats_g, in_=stats_out[:, :])

        out_sb = small.tile([128, CCH], F32, tag="outsb")
        mean2 = small.tile([128, CCH], F32, tag="fin")
        e22 = small.tile([128, CCH], F32, tag="fin")
        var2 = small.tile([128, CCH], F32, tag="fin")
        nc.vector.tensor_scalar_mul(out=mean2, in0=stats_g[:, 0:2],
                                    scalar1=INV_CNT)
        nc.vector.tensor_scalar_mul(out=e22, in0=stats_g[:, 2:4],
                                    scalar1=INV_CNT)
        m22 = small.tile([128, CCH], F32, tag="fin")
        nc.scalar.square(out=m22, in_=mean2)
        nc.vector.tensor_tensor(out=var2, in0=e22, in1=m22, op=ALU.subtract)
        sd2 = small.tile([128, CCH], F32, tag="fin")
        nc.scalar.activation(out=sd2, in_=var2, func=AF.Sqrt, bias=eps_sb,
                             scale=1.0)
        rstd2 = small.tile([128, CCH], F32, tag="fin")
        nc.vector.reciprocal_approx_fast(out=rstd2, in_=sd2)
        scale2 = small.tile([128, CCH], F32, tag="fin")
        nc.vector.tensor_tensor(out=scale2, in0=rstd2, in1=gamma_sb,
                                op=ALU.mult)
        ms2 = small.tile([128, CCH], F32, tag="fin")
        nc.vector.tensor_tensor(out=ms2, in0=mean2, in1=scale2, op=ALU.mult)
        negshift2 = small.tile([128, CCH], F32, tag="fin")
        nc.vector.tensor_tensor(out=negshift2, in0=ms2, in1=beta_sb,
                                op=ALU.subtract)

        for ch in range(CCH):
            # z = wy16*scale + x16 ; out = max_n z - negshift
            z = sm.tile([128, N], F16, tag=f"z{ch}")
            nc.vector.scalar_tensor_tensor(out=z, in0=wy16[ch][:, :],
                                           scalar=scale2[:, ch:ch + 1],
                                           in1=x16[ch][:, :], op0=ALU.mult,
                                           op1=ALU.add)
            mx = small.tile([128, 1], F16, tag="finh")
            nc.vector.tensor_reduce(out=mx, in_=z, axis=AX.X, op=ALU.max)
            nc.vector.tensor_tensor(out=out_sb[:, ch:ch + 1], in0=mx,
                                    in1=negshift2[:, ch:ch + 1],
                                    op=ALU.subtract)
            nc.gpsimd.dma_start(out=out_d[ch, :].rearrange("(p one) -> p one", one=1),
                                in_=out_sb[:, ch:ch + 1])

    nc.compile()
    return nc


_LAST = {}


def _to_bf16(a):
    try:
        import ml_dtypes
        return np.ascontiguousarray(a.astype(ml_dtypes.bfloat16))
    except ImportError:
        import jax.numpy as jnp
        return np.ascontiguousarray(np.asarray(jnp.asarray(a, dtype=jnp.bfloat16)))


def kernel(**inputs):
    x = np.ascontiguousarray(inputs["x"], dtype=np.float32)      # (8, 256, 64, 64)
    Wg = np.asarray(inputs["Wg"], dtype=np.float32)
    bg = np.asarray(inputs["bg"], dtype=np.float32)
    Wt = np.asarray(inputs["Wt"], dtype=np.float32)
    bt = np.asarray(inputs["bt"], dtype=np.float32)
    Wp = np.asarray(inputs["Wp"], dtype=np.float32)
    bp = np.asarray(inputs["bp"], dtype=np.float32)
    Ww = np.asarray(inputs["Ww"], dtype=np.float32)
    bw = np.asarray(inputs["bw"], dtype=np.float32)
    gamma = np.asarray(inputs["gamma"], dtype=np.float32)
    beta = np.asarray(inputs["beta"], dtype=np.float32)

    if "nc" not in _CACHE:
        _CACHE["nc"] = _build()
    nc = _CACHE["nc"]

    shared = {
        "WtT": _to_bf16(np.ascontiguousarray(Wt.T)),
        "WpT": _to_bf16(np.ascontiguousarray(Wp.T)),
        "WgT": _to_bf16(np.ascontiguousarray(Wg.T)),
        "WwT": _to_bf16(np.ascontiguousarray(Ww.T)),
        "smalls": np.ascontiguousarray(np.concatenate([
            bt.reshape(CI, 1), bp.reshape(CI, 1), bg.reshape(CI, 1),
            gamma.reshape(CCH, 128).T, beta.reshape(CCH, 128).T,
        ], axis=1).astype(np.float32)),
        "ones_k": _to_bf16(np.ones((128, 1), dtype=np.float32)),
        "ones_p": _to_bf16(np.ones((1, 128), dtype=np.float32)),
    }
    in_maps = [dict(shared, x=_to_bf16(x[b].reshape(C, N)))
               for b in range(B)]
    import os
    trace = bool(int(os.environ.get("KERNEL_TRACE", "0")))
    res = run_bass_kernel_spmd(nc, in_maps, core_ids=list(range(B)), trace=trace)
    _LAST["res"] = res
    out = np.stack([np.asarray(res.results[b]["out"]).reshape(C) for b in range(B)])
    return out.reshape(B, C, 1, 1).astype(np.float32)


if __name__ == "__main__":
    pass
